# revision 1
# baseline (speedup 1.0000x reference)
"""Distributed spectral conv on S2 (SHT -> per-l complex channel mix -> ISHT)
for Trainium2, m-mode sharded across 8 NeuronCores.

v3 design (per core, 34 m-slots: <=17 even-parity + <=17 odd-parity modes,
zero-padded; 8*33 real modes cover 257):
  A: DFT over lon, x-chunks as PE weights, bf16    -> XFT[kp,(par,mi,comp,c)]
  B: parity-folded Legendre, XFT slices as weights -> CFQ_s[i, j*68+{R,I}]
  C: 4-quadrant complex mix, one FWL matmul/(s,j)  -> COUT4[(h,o),(s,mi,j)]
  P1: PE transpose                                 -> OUTT[j,(s,mi,h,o)]
  D: inverse Legendre, e/o fold in psum accum      -> XKS[kp,(kh,o,mc)]
  P2: PE transpose                                 -> XK[mc,(kh,o,kp)]
  E: inverse DFT bf16                              -> y_part[(kh,o,kp), n]
Host sums 8 partial outputs; kh=1 rows are latitude-reversed (k=255-kp).
"""
import numpy as np
import ml_dtypes

import concourse.bass as bass
import concourse.bacc as bacc
import concourse.mybir as mybir
from concourse import tile
from concourse._compat import get_trn_type
from concourse.bass_utils import run_bass_kernel_spmd

F32 = mybir.dt.float32
BF16 = mybir.dt.bfloat16

N_CORES = 8
M_RAW = 33
M_LOC = 34            # slot count: 17 even-parity + 17 odd-parity
NE = 17
MC = 2 * M_LOC        # 68
CIN = 64
COUT_ = 64
NLAT = 256
NLON = 512
MMAX = 257
KP = 128

_prog_cache = {}


def _build_nc():
    nc = bacc.Bacc(get_trn_type() or "TRN2", target_bir_lowering=False, debug=False)

    xt2 = nc.dram_tensor("xt2", [2, 4, 128, CIN * KP], BF16, kind="ExternalInput")
    fdft = nc.dram_tensor("fdft", [4, 128, MC], BF16, kind="ExternalInput")
    shtw = nc.dram_tensor("shtw", [M_LOC, KP, 2, 128], BF16, kind="ExternalInput")
    wc = nc.dram_tensor("wc", [64, 2, 128, 2, 128], BF16, kind="ExternalInput")
    pctw = nc.dram_tensor("pctw", [M_LOC, 128, 4, KP], BF16, kind="ExternalInput")
    gdft = nc.dram_tensor("gdft", [MC, NLON], BF16, kind="ExternalInput")
    ident = nc.dram_tensor("ident", [128, 128], BF16, kind="ExternalInput")
    y_part = nc.dram_tensor("y_part", [2 * 64 * KP, NLON], BF16, kind="ExternalOutput")

    with tile.TileContext(nc) as tc:
        with tc.tile_pool(name="const", bufs=1) as constp, \
             tc.tile_pool(name="big", bufs=1) as bigp, \
             tc.tile_pool(name="xa", bufs=3) as xap, \
             tc.tile_pool(name="sw", bufs=2) as swp, \
             tc.tile_pool(name="wt", bufs=2) as wtp, \
             tc.tile_pool(name="pt", bufs=2) as ptp, \
             tc.tile_pool(name="ysb", bufs=6) as ysbp, \
             tc.tile_pool(name="ps", bufs=4, space="PSUM") as psp, \
             tc.tile_pool(name="pst", bufs=4, space="PSUM") as pstp:

            fsbr = constp.tile([128, 4, MC], BF16)      # [ni, nc4, cm]
            gsb = constp.tile([MC, NLON], BF16)
            isbb = constp.tile([128, 128], BF16)
            nc.sync.dma_start(fsbr[:, :, :], fdft.ap().rearrange("a b c -> b a c"))
            nc.sync.dma_start(gsb[:, :], gdft[:, :])
            nc.sync.dma_start(isbb[:, :], ident[:, :])

            # ---- stage A: DFT as matmul, x chunks as weights ----
            XFT = bigp.tile([128, 2 * M_LOC * 128], BF16, tag="bigA")
            XFT_v = XFT.rearrange("p (par mi comp c) -> p par mi comp c",
                                  par=2, comp=2, c=64)
            ei = 0
            for par in range(2):
                for sp in range(8):                 # 8 c's per 1024-col span
                    xa = xap.tile([128, 4, 1024], BF16, tag="xa")
                    nc.sync.dma_start(
                        xa[:, :, :],
                        xt2.ap()[par, :, :, sp * 1024:(sp + 1) * 1024]
                        .rearrange("a b c -> b a c")
                    )
                    for g in range(2):              # psum groups of 4 c's
                        pa = psp.tile([128, 4 * MC], F32, tag="ps")
                        for cl in range(4):
                            cc = g * 4 + cl
                            for nc4 in range(4):
                                nc.tensor.matmul(
                                    pa[:, cl * MC:(cl + 1) * MC],
                                    xa[:, nc4, cc * 128:(cc + 1) * 128],
                                    fsbr[:, nc4, :],
                                    start=(nc4 == 0),
                                    stop=(nc4 == 3),
                                )
                        c0 = sp * 8 + g * 4
                        dst = XFT_v[:, par, :, :, c0:c0 + 4].rearrange(
                            "p mi comp cl -> p cl comp mi")
                        src = pa.rearrange("p (cl comp mi) -> p cl comp mi",
                                           cl=4, comp=2)
                        if ei % 2 == 0:
                            nc.vector.tensor_copy(dst, src)
                        else:
                            nc.scalar.copy(dst, src)
                        ei += 1

            # ---- stage B: folded Legendre, XFT slices as weights ----
            # j-blocked layout: cols (jg 8, cm 68, jl 16) -> 32B runs for the
            # B evacuation; C reads rhs cols at 32B stride
            CFQ0 = bigp.tile([64, 128 * MC], BF16, tag="bigB0")
            CFQ1 = bigp.tile([64, 128 * MC], BF16, tag="bigB1")
            CFQ = [CFQ0, CFQ1]
            CFQ_v = [t.rearrange("p (jg cm jl) -> p jg cm jl", jg=8, jl=16)
                     for t in CFQ]
            # mi pair-units never straddle the parity groups (0..16, 17..33);
            # swr DMA chunks aligned to unit boundaries so B starts early
            units = [(mi, 2) for mi in range(0, 16, 2)] + [(16, 1)] + \
                    [(mi, 2) for mi in range(17, 33, 2)] + [(33, 1)]
            chunks = [(0, units[0:4]), (8, units[4:8]), (16, units[8:12]),
                      (23, units[12:16]), (31, units[16:18])]
            ui = 0
            for mi0c, chunk_units in chunks:
                nmi = sum(nu for _, nu in chunk_units)
                swr = swp.tile([128, 8, 2, 128], BF16, tag="sw")
                nc.sync.dma_start(
                    swr[:, 0:nmi, :, :],
                    shtw.ap()[mi0c:mi0c + nmi].rearrange("m k p j -> k m p j")
                )
                for mi0, nu in chunk_units:
                    pb = psp.tile([128, 512], F32, tag="ps")
                    for ml in range(nu):
                        for par in range(2):
                            nc.tensor.matmul(
                                pb[:, ml * 256 + par * 128:
                                   ml * 256 + (par + 1) * 128],
                                XFT_v[:, par, mi0 + ml]
                                .rearrange("p comp c -> p (comp c)"),
                                swr[:, mi0 - mi0c + ml, par, :],
                                start=True, stop=True,
                            )
                    blk0 = 0 if mi0 < NE else 1
                    for par in range(2):
                        blk = (blk0 + par) % 2
                        dR = CFQ_v[blk][:, :, mi0:mi0 + nu, :] \
                            .rearrange("p jg u jl -> p u jg jl")
                        dI = CFQ_v[blk][:, :, M_LOC + mi0:M_LOC + mi0 + nu, :] \
                            .rearrange("p jg u jl -> p u jg jl")
                        sv = pb.rearrange("p (u par jg jl) -> p u par jg jl",
                                          par=2, jg=8, jl=16)
                        sR = sv[0:64, 0:nu, par, :, :]
                        sI = sv[64:128, 0:nu, par, :, :]
                        if (ui + par) % 2 == 0:
                            nc.vector.tensor_copy(dR, sR)
                            nc.scalar.copy(dI, sI)
                        else:
                            nc.scalar.copy(dR, sR)
                            nc.vector.tensor_copy(dI, sI)
                    ui += 1

            # ---- stage C: complex mix, combine folded into psum accum ----
            # per (s, j): psum[(h,o), mi] = W_a^T R + W_b^T I with
            # W_a = [wr | wi], W_b = [-wi | wr] stacked over (h,o) cols
            COUT4 = bigp.tile([128, 2 * M_LOC * 128], BF16, tag="bigC")
            C4_v = COUT4.rearrange("p (s mi j) -> p s mi j", s=2, j=128)
            for s in range(2):
                for jc in range(16):                # 8 j per weight chunk
                    wt = wtp.tile([64, 8, 2, 128], BF16, tag="wt")
                    nc.sync.dma_start(
                        wt[:, :, :, :],
                        wc.ap()[:, s, jc * 8:(jc + 1) * 8]
                    )
                    pc = psp.tile([128, 8 * M_LOC], F32, tag="ps")
                    for jx in range(8):
                        j = jc * 8 + jx
                        jg, jl = j // 16, j % 16
                        nc.tensor.matmul(
                            pc[:, jx * M_LOC:(jx + 1) * M_LOC],
                            wt[:, jx, 0, :],
                            CFQ_v[s][:, jg, 0:M_LOC, jl],
                            start=True, stop=False,
                        )
                        nc.tensor.matmul(
                            pc[:, jx * M_LOC:(jx + 1) * M_LOC],
                            wt[:, jx, 1, :],
                            CFQ_v[s][:, jg, M_LOC:MC, jl],
                            start=False, stop=True,
                        )
                    dst = C4_v[:, s, :, jc * 8:(jc + 1) * 8] \
                        .rearrange("p mi jl -> p jl mi")
                    src = pc.rearrange("p (jl mi) -> p jl mi", jl=8)
                    if jc % 2 == 0:
                        nc.vector.tensor_copy(dst, src)
                    else:
                        nc.scalar.copy(dst, src)

            # ---- pivot P1: COUT4 -> OUTT[j, (s,mi,h,o)] via PE transpose ----
            OUTT = bigp.tile([128, 2 * M_LOC * 128], BF16, tag="bigD")
            smi = [(s, mi) for s in range(2) for mi in range(M_LOC)]
            for tg in range(17):                    # 4 transposes per group
                pieces = smi[tg * 4:tg * 4 + 4]
                pt1 = pstp.tile([128, 512], BF16, tag="pst")
                for pi, (s, mi) in enumerate(pieces):
                    nc.tensor.transpose(
                        pt1[:, pi * 128:(pi + 1) * 128],
                        C4_v[:, s, mi, :], isbb[:, :]
                    )
                base = tg * 512
                if tg % 2 == 0:
                    nc.scalar.copy(OUTT[:, base:base + 512], pt1[:, :])
                else:
                    nc.vector.tensor_copy(OUTT[:, base:base + 512], pt1[:, :])

            # ---- stage D: inverse Legendre, e/o fold in psum ----
            XKS = bigp.tile([128, 2 * 64 * MC], BF16, tag="bigA")
            XKS_v = XKS.rearrange("p (kh o mc) -> p kh o mc", kh=2, o=64)
            OUTT_v = OUTT.rearrange("p (s mi x) -> p s mi x", s=2, x=128)
            for mc8 in range(5):
                mi0 = mc8 * 8
                nmi = min(8, M_LOC - mi0)
                pt = ptp.tile([128, 8, 4, KP], BF16, tag="pt")
                nc.sync.dma_start(
                    pt[:, 0:nmi, :, :],
                    pctw.ap()[mi0:mi0 + nmi].rearrange("m j v k -> j m v k")
                )
                for ml in range(nmi):
                    mi = mi0 + ml
                    pd = psp.tile([128, 256], F32, tag="ps")
                    for kh in range(2):
                        nc.tensor.matmul(
                            pd[:, kh * 128:(kh + 1) * 128],
                            pt[:, ml, 2 * kh, :], OUTT_v[:, 0, mi, :],
                            start=True, stop=False,
                        )
                        nc.tensor.matmul(
                            pd[:, kh * 128:(kh + 1) * 128],
                            pt[:, ml, 2 * kh + 1, :], OUTT_v[:, 1, mi, :],
                            start=False, stop=True,
                        )
                    # psum cols are (o,h) [host weight order] -> src is
                    # contiguous and dst has 4B (h-pair) runs
                    dv = XKS_v[:, :, :, 2 * mi:2 * mi + 2]
                    sv = pd.rearrange("p (kh o h) -> p kh o h", kh=2, h=2)
                    if mi % 2 == 0:
                        nc.scalar.copy(dv, sv)
                    else:
                        nc.vector.tensor_copy(dv, sv)

            # ---- P2 + E interleaved: transpose 4 blocks, then 2 E pairs ----
            XK = bigp.tile([MC, 2 * 64 * 128], BF16, tag="bigE")
            for tg in range(32):
                pt2 = pstp.tile([MC, 512], BF16, tag="pst")
                for pi in range(4):
                    blk = tg * 4 + pi               # blk = kh*64 + o
                    kh, o = blk // 64, blk % 64
                    nc.tensor.transpose(
                        pt2[:, pi * 128:(pi + 1) * 128],
                        XKS_v[:, kh, o, :], isbb[:, :]
                    )
                if tg % 2 == 0:
                    nc.scalar.copy(XK[:, tg * 512:(tg + 1) * 512], pt2[:, :])
                else:
                    nc.vector.tensor_copy(XK[:, tg * 512:(tg + 1) * 512],
                                          pt2[:, :])
                for jp in (2 * tg, 2 * tg + 1):     # E over blocks just built
                    ys = ysbp.tile([128, 2, NLON], BF16, tag="ys2")
                    for h in range(2):
                        blk = 2 * jp + h
                        pe = psp.tile([128, NLON], F32, tag="ps")
                        nc.tensor.matmul(
                            pe[:, :], XK[:, blk * 128:(blk + 1) * 128],
                            gsb[:, :], start=True, stop=True,
                        )
                        if h == 0:
                            nc.vector.tensor_copy(ys[:, h, :], pe[:, :])
                        else:
                            nc.scalar.copy(ys[:, h, :], pe[:, :])
                    nc.sync.dma_start(
                        y_part.ap()[jp * 256:(jp + 1) * 256, :]
                        .rearrange("(a p) n -> p a n", a=2),
                        ys[:, :, :],
                    )

    return nc


def _get_nc():
    if "v3" not in _prog_cache:
        nc = _build_nc()
        nc.compile()
        _prog_cache["v3"] = nc
    return _prog_cache["v3"]


def _core_slots(r):
    """slot -> m (or None). slots 0..16: even-parity m ascending; 17..: odd."""
    ms = [r * M_RAW + t for t in range(M_RAW) if r * M_RAW + t < MMAX]
    ev = [m for m in ms if m % 2 == 0]
    od = [m for m in ms if m % 2 == 1]
    slots = [None] * M_LOC
    for i, m in enumerate(ev):
        slots[i] = m
    for i, m in enumerate(od):
        slots[NE + i] = m
    return slots


def make_in_maps(x, weight_r, weight_i, pct, sht_w):
    BF = ml_dtypes.bfloat16
    x = np.asarray(x, np.float32)[0]                # [c, k, n]
    wr = np.asarray(weight_r, np.float32)[0]        # [i, o, l]
    wi = np.asarray(weight_i, np.float32)[0]
    pct = np.asarray(pct, np.float32)               # [m, l, k]
    sht_w = np.asarray(sht_w, np.float32)

    xr = x[:, ::-1, :][:, :KP, :]
    xe = x[:, :KP, :] + xr
    xo = x[:, :KP, :] - xr
    xt2 = np.stack([
        xp.transpose(2, 0, 1).reshape(NLON, CIN * KP).reshape(4, 128, CIN * KP)
        for xp in (xe, xo)]).astype(BF)

    # W_a = [wr ; wi], W_b = [-wi ; wr] interleaved over (o,h) cols, per (s,j)
    wcm = np.zeros((64, 2, 128, 2, 128), np.float32)
    for s in range(2):
        wl = wr[:, :, s::2].transpose(0, 2, 1)      # [i, j, o]
        il = wi[:, :, s::2].transpose(0, 2, 1)
        wcm[:, s, :, 0, :] = np.stack([wl, il], axis=-1).reshape(64, 128, 128)
        wcm[:, s, :, 1, :] = np.stack([-il, wl], axis=-1).reshape(64, 128, 128)
    wcm = wcm.astype(BF)

    identb = np.eye(128, dtype=np.float32).astype(BF)
    n = np.arange(NLON)
    js = np.arange(128)

    in_maps = []
    for r in range(N_CORES):
        slots = _core_slots(r)
        fdft = np.zeros((NLON, MC), np.float32)
        gdft = np.zeros((MC, NLON), np.float32)
        shtw_p = np.zeros((M_LOC, KP, 2, 128), np.float32)
        pctw_p = np.zeros((M_LOC, 128, 4, KP), np.float32)
        for mi, m in enumerate(slots):
            if m is None:
                continue
            ang = 2.0 * np.pi * m * n / NLON
            fdft[:, mi] = (2.0 * np.pi / NLON) * np.cos(ang)
            fdft[:, M_LOC + mi] = -(2.0 * np.pi / NLON) * np.sin(ang)
            cmf = 1.0 if (m == 0 or m == NLON // 2) else 2.0
            gdft[2 * mi] = cmf * np.cos(ang)
            gdft[2 * mi + 1] = -cmf * np.sin(ang)
            for par in range(2):
                cls = (m + par) % 2     # l-parity class in psum half `par`
                shtw_p[mi, :, par, :] = sht_w[m, 2 * js + cls, :KP].T
            sgn = 1.0 if m % 2 == 0 else -1.0
            pctw_p[mi, :, 0, :] = pct[m, 2 * js, :KP]
            pctw_p[mi, :, 1, :] = pct[m, 2 * js + 1, :KP]
            pctw_p[mi, :, 2, :] = sgn * pct[m, 2 * js, :KP]
            pctw_p[mi, :, 3, :] = -sgn * pct[m, 2 * js + 1, :KP]

        in_maps.append({
            "xt2": xt2, "fdft": fdft.reshape(4, 128, MC).astype(BF),
            "shtw": shtw_p.astype(BF), "wc": wcm,
            "pctw": pctw_p.astype(BF),
            "gdft": gdft.astype(BF), "ident": identb,
        })
    return in_maps


def kernel(x, weight_r, weight_i, pct, sht_w):
    x_np = np.asarray(x)
    nc = _get_nc()
    in_maps = make_in_maps(x_np, weight_r, weight_i, pct, sht_w)
    try:
        res = run_bass_kernel_spmd(nc, in_maps, list(range(N_CORES)))
    except Exception:
        # transient NRT exec faults have been observed on the first run
        # after a NEFF load; one retry has always succeeded
        res = run_bass_kernel_spmd(nc, in_maps, list(range(N_CORES)))
    y = np.zeros((64, NLAT, NLON), np.float64)
    for r in range(N_CORES):
        yp = np.asarray(res.results[r]["y_part"], dtype=np.float64)
        yp = yp.reshape(2, 64, KP, NLON)
        y[:, :KP, :] += yp[0]
        y[:, KP:, :] += yp[1][:, ::-1, :]
    y = y.astype(np.float32).reshape(1, COUT_, NLAT, NLON)
    return (y, x_np)



# revision 2
# speedup vs baseline: 1.3140x; 1.3140x over previous
"""Distributed spectral conv on S2 (SHT -> per-l complex channel mix -> ISHT)
for Trainium2, m-mode sharded across 8 NeuronCores.

v4 design (per core, 34 m-slots: <=17 even-parity + <=17 odd-parity modes,
zero-padded; 8*33 real modes cover 257):
  A: DFT over lon, x-chunks as PE weights, bf16    -> XFT[kp,(par,mi,comp,c)]
  B: parity-folded Legendre, XFT slices as weights -> CFQ2_s[(comp,i), (jg,m,jl)]
  C: complex mix as ONE K=128 matmul/(s,j): stat=[[wr|wi];[-wi|wr]] stacked
     over (R,I) rows, mov = CFQ2 slice           -> COUT4[(h,o),(s,mi,j)]
  P1: PE transpose                                 -> OUTT[j,(s,mi,h,o)]
  D: inverse Legendre, e/o fold in psum accum      -> XKS[kp,(kh,o,mc)]
  P2: PE transpose (all blocks up front)           -> XK[mc,(kh,o,kp)]
  E: inverse DFT bf16, dense back-to-back MMs      -> y_part[(kh,o,kp), n]
Host sums 8 partial outputs; kh=1 rows are latitude-reversed (k=255-kp).
"""
import numpy as np
import ml_dtypes

import concourse.bass as bass
import concourse.bacc as bacc
import concourse.mybir as mybir
from concourse import tile
from concourse._compat import get_trn_type
from concourse.bass_utils import run_bass_kernel_spmd

F32 = mybir.dt.float32
BF16 = mybir.dt.bfloat16

N_CORES = 8
M_RAW = 33
M_LOC = 34            # slot count: 17 even-parity + 17 odd-parity
NE = 17
MC = 2 * M_LOC        # 68
CIN = 64
COUT_ = 64
NLAT = 256
NLON = 512
MMAX = 257
KP = 128

_prog_cache = {}


def _build_nc():
    nc = bacc.Bacc(get_trn_type() or "TRN2", target_bir_lowering=False, debug=False)

    xt2 = nc.dram_tensor("xt2", [2, 4, 128, CIN * KP], BF16, kind="ExternalInput")
    fdft = nc.dram_tensor("fdft", [4, 128, MC], BF16, kind="ExternalInput")
    shtw = nc.dram_tensor("shtw", [M_LOC, KP, 2, 128], BF16, kind="ExternalInput")
    wc = nc.dram_tensor("wc", [2, 128, 128, 128], BF16, kind="ExternalInput")
    pctw = nc.dram_tensor("pctw", [M_LOC, 128, 4, KP], BF16, kind="ExternalInput")
    gdft = nc.dram_tensor("gdft", [MC, NLON], BF16, kind="ExternalInput")
    ident = nc.dram_tensor("ident", [128, 128], BF16, kind="ExternalInput")
    y_part = nc.dram_tensor("y_part", [2 * 64 * KP, NLON], BF16, kind="ExternalOutput")

    with tile.TileContext(nc) as tc:
        with tc.tile_pool(name="const", bufs=1) as constp, \
             tc.tile_pool(name="big", bufs=1) as bigp, \
             tc.tile_pool(name="xa", bufs=3) as xap, \
             tc.tile_pool(name="wt", bufs=4) as wtp, \
             tc.tile_pool(name="pt", bufs=2) as ptp, \
             tc.tile_pool(name="ysb", bufs=6) as ysbp, \
             tc.tile_pool(name="ps", bufs=4, space="PSUM") as psp, \
             tc.tile_pool(name="pst", bufs=4, space="PSUM") as pstp:

            fsbr = constp.tile([128, 4, MC], BF16)      # [ni, nc4, cm]
            gsb = constp.tile([MC, NLON], BF16)
            isbb = constp.tile([128, 128], BF16)
            swr = constp.tile([128, M_LOC, 2, 128], BF16)   # all Legendre wts
            nc.sync.dma_start(fsbr[:, :, :], fdft.ap().rearrange("a b c -> b a c"))
            nc.sync.dma_start(isbb[:, :], ident[:, :])
            nc.sync.dma_start(gsb[:, :], gdft[:, :])
            nc.sync.dma_start(
                swr[:, :, :, :], shtw.ap().rearrange("m k p j -> k m p j"))

            # ---- stage A: DFT as matmul, x chunks as weights ----
            XFT = bigp.tile([128, 2 * M_LOC * 128], BF16, tag="bigA")
            XFT_v = XFT.rearrange("p (par mi comp c) -> p par mi comp c",
                                  par=2, comp=2, c=64)
            ei = 0
            for par in range(2):
                for sp in range(8):                 # 8 c's per 1024-col span
                    xa = xap.tile([128, 4, 1024], BF16, tag="xa")
                    nc.sync.dma_start(
                        xa[:, :, :],
                        xt2.ap()[par, :, :, sp * 1024:(sp + 1) * 1024]
                        .rearrange("a b c -> b a c")
                    )
                    for g in range(2):              # psum groups of 4 c's
                        pa = psp.tile([128, 4 * MC], F32, tag="ps")
                        for cl in range(4):
                            cc = g * 4 + cl
                            for nc4 in range(4):
                                nc.tensor.matmul(
                                    pa[:, cl * MC:(cl + 1) * MC],
                                    xa[:, nc4, cc * 128:(cc + 1) * 128],
                                    fsbr[:, nc4, :],
                                    start=(nc4 == 0),
                                    stop=(nc4 == 3),
                                )
                        c0 = sp * 8 + g * 4
                        dst = XFT_v[:, par, :, :, c0:c0 + 4].rearrange(
                            "p mi comp cl -> p cl comp mi")
                        src = pa.rearrange("p (cl comp mi) -> p cl comp mi",
                                           cl=4, comp=2)
                        if ei % 2 == 0:
                            nc.vector.tensor_copy(dst, src)
                        else:
                            nc.scalar.copy(dst, src)
                        ei += 1

            # ---- stage B: folded Legendre, XFT slices as weights ----
            # CFQ2_s layout: partitions (comp 2, i 64); cols (jg 8, m 34, jl 16)
            CFQ0 = bigp.tile([128, M_LOC * 128], BF16, tag="bigB0")
            CFQ1 = bigp.tile([128, M_LOC * 128], BF16, tag="bigB1")
            CFQ_v = [t.rearrange("p (jg m jl) -> p jg m jl", jg=8, jl=16)
                     for t in (CFQ0, CFQ1)]
            # mi pair-units never straddle the parity groups (0..16, 17..33)
            units = [(mi, 2) for mi in range(0, 16, 2)] + [(16, 1)] + \
                    [(mi, 2) for mi in range(17, 33, 2)] + [(33, 1)]
            ui = 0
            for mi0, nu in units:
                pb = psp.tile([128, 512], F32, tag="ps")
                for ml in range(nu):
                    for par in range(2):
                        nc.tensor.matmul(
                            pb[:, ml * 256 + par * 128:
                               ml * 256 + (par + 1) * 128],
                            XFT_v[:, par, mi0 + ml]
                            .rearrange("p comp c -> p (comp c)"),
                            swr[:, mi0 + ml, par, :],
                            start=True, stop=True,
                        )
                blk0 = 0 if mi0 < NE else 1
                sv = pb.rearrange("p (u par jg jl) -> p u par jg jl",
                                  par=2, jg=8, jl=16)
                for par in range(2):
                    blk = (blk0 + par) % 2
                    dst = CFQ_v[blk][:, :, mi0:mi0 + nu, :] \
                        .rearrange("p jg u jl -> p u jg jl")
                    if (ui + par) % 2 == 0:
                        nc.vector.tensor_copy(dst, sv[:, 0:nu, par])
                    else:
                        nc.scalar.copy(dst, sv[:, 0:nu, par])
                ui += 1

            # ---- stage C: complex mix, ONE K=128 matmul per (s,j) ----
            # stat rows 0:64 = [wr|wi] (vs R), rows 64:128 = [-wi|wr] (vs I)
            COUT4 = bigp.tile([128, 2 * M_LOC * 128], BF16, tag="bigC")
            C4_v = COUT4.rearrange("p (s mi j) -> p s mi j", s=2, j=128)
            for s in range(2):
                for jh in range(8):                 # 16 j per weight chunk
                    wt = wtp.tile([128, 16, 128], BF16, tag="wt")
                    nc.sync.dma_start(
                        wt[:, :, :],
                        wc.ap()[s, jh * 16:(jh + 1) * 16]
                        .rearrange("j p c -> p j c")
                    )
                    for jc in (2 * jh, 2 * jh + 1):
                        pc = psp.tile([128, 8 * M_LOC], F32, tag="ps")
                        for jx in range(8):
                            j = jc * 8 + jx
                            jg, jl = j // 16, j % 16
                            nc.tensor.matmul(
                                pc[:, jx * M_LOC:(jx + 1) * M_LOC],
                                wt[:, j - jh * 16, :],
                                CFQ_v[s][:, jg, :, jl],
                                start=True, stop=True,
                            )
                        dst = C4_v[:, s, :, jc * 8:(jc + 1) * 8] \
                            .rearrange("p mi jl -> p jl mi")
                        src = pc.rearrange("p (jl mi) -> p jl mi", jl=8)
                        if jc % 2 == 0:
                            nc.vector.tensor_copy(dst, src)
                        else:
                            nc.scalar.copy(dst, src)

            # ---- pivot P1: COUT4 -> OUTT[j, (s,mi,h,o)] via PE transpose ----
            OUTT = bigp.tile([128, 2 * M_LOC * 128], BF16, tag="bigD")
            smi = [(s, mi) for s in range(2) for mi in range(M_LOC)]
            for tg in range(17):                    # 4 transposes per group
                pieces = smi[tg * 4:tg * 4 + 4]
                pt1 = pstp.tile([128, 512], BF16, tag="pst")
                for pi, (s, mi) in enumerate(pieces):
                    nc.tensor.transpose(
                        pt1[:, pi * 128:(pi + 1) * 128],
                        C4_v[:, s, mi, :], isbb[:, :]
                    )
                base = tg * 512
                if tg % 2 == 0:
                    nc.scalar.copy(OUTT[:, base:base + 512], pt1[:, :])
                else:
                    nc.vector.tensor_copy(OUTT[:, base:base + 512], pt1[:, :])

            # ---- stage D: inverse Legendre, e/o fold in psum ----
            XKS = bigp.tile([128, 2 * 64 * MC], BF16, tag="bigA")
            XKS_v = XKS.rearrange("p (kh o mc) -> p kh o mc", kh=2, o=64)
            OUTT_v = OUTT.rearrange("p (s mi x) -> p s mi x", s=2, x=128)
            for mc8 in range(5):
                mi0 = mc8 * 8
                nmi = min(8, M_LOC - mi0)
                pt = ptp.tile([128, 8, 4, KP], BF16, tag="pt")
                nc.sync.dma_start(
                    pt[:, 0:nmi, :, :],
                    pctw.ap()[mi0:mi0 + nmi].rearrange("m j v k -> j m v k")
                )
                for ml in range(nmi):
                    mi = mi0 + ml
                    pd = psp.tile([128, 256], F32, tag="ps")
                    for kh in range(2):
                        nc.tensor.matmul(
                            pd[:, kh * 128:(kh + 1) * 128],
                            pt[:, ml, 2 * kh, :], OUTT_v[:, 0, mi, :],
                            start=True, stop=False,
                        )
                        nc.tensor.matmul(
                            pd[:, kh * 128:(kh + 1) * 128],
                            pt[:, ml, 2 * kh + 1, :], OUTT_v[:, 1, mi, :],
                            start=False, stop=True,
                        )
                    # psum cols are (o,h) [host weight order] -> src is
                    # contiguous and dst has 4B (h-pair) runs
                    dv = XKS_v[:, :, :, 2 * mi:2 * mi + 2]
                    sv = pd.rearrange("p (kh o h) -> p kh o h", kh=2, h=2)
                    if mi % 2 == 0:
                        nc.scalar.copy(dv, sv)
                    else:
                        nc.vector.tensor_copy(dv, sv)

            # ---- P2: transpose ALL blocks first (keeps E dense/warm) ----
            XK = bigp.tile([MC, 2 * 64 * 128], BF16, tag="bigE")
            for tg in range(32):
                pt2 = pstp.tile([MC, 512], BF16, tag="pst")
                for pi in range(4):
                    blk = tg * 4 + pi               # blk = kh*64 + o
                    kh, o = blk // 64, blk % 64
                    nc.tensor.transpose(
                        pt2[:, pi * 128:(pi + 1) * 128],
                        XKS_v[:, kh, o, :], isbb[:, :]
                    )
                if tg % 2 == 0:
                    nc.scalar.copy(XK[:, tg * 512:(tg + 1) * 512], pt2[:, :])
                else:
                    nc.vector.tensor_copy(XK[:, tg * 512:(tg + 1) * 512],
                                          pt2[:, :])

            # ---- stage E: dense back-to-back inverse-DFT matmuls ----
            for jp in range(64):
                ys = ysbp.tile([128, 2, NLON], BF16, tag="ys2")
                for h in range(2):
                    blk = 2 * jp + h
                    pe = psp.tile([128, NLON], F32, tag="ps")
                    nc.tensor.matmul(
                        pe[:, :], XK[:, blk * 128:(blk + 1) * 128],
                        gsb[:, :], start=True, stop=True,
                    )
                    if h == 0:
                        nc.vector.tensor_copy(ys[:, h, :], pe[:, :])
                    else:
                        nc.scalar.copy(ys[:, h, :], pe[:, :])
                nc.sync.dma_start(
                    y_part.ap()[jp * 256:(jp + 1) * 256, :]
                    .rearrange("(a p) n -> p a n", a=2),
                    ys[:, :, :],
                )

    return nc


def _get_nc():
    if "v4" not in _prog_cache:
        nc = _build_nc()
        nc.compile()
        _prog_cache["v4"] = nc
    return _prog_cache["v4"]


def _core_slots(r):
    """slot -> m (or None). slots 0..16: even-parity m ascending; 17..: odd."""
    ms = [r * M_RAW + t for t in range(M_RAW) if r * M_RAW + t < MMAX]
    ev = [m for m in ms if m % 2 == 0]
    od = [m for m in ms if m % 2 == 1]
    slots = [None] * M_LOC
    for i, m in enumerate(ev):
        slots[i] = m
    for i, m in enumerate(od):
        slots[NE + i] = m
    return slots


def make_in_maps(x, weight_r, weight_i, pct, sht_w):
    BF = ml_dtypes.bfloat16
    x = np.asarray(x, np.float32)[0]                # [c, k, n]
    wr = np.asarray(weight_r, np.float32)[0]        # [i, o, l]
    wi = np.asarray(weight_i, np.float32)[0]
    pct = np.asarray(pct, np.float32)               # [m, l, k]
    sht_w = np.asarray(sht_w, np.float32)

    xr = x[:, ::-1, :][:, :KP, :]
    xe = x[:, :KP, :] + xr
    xo = x[:, :KP, :] - xr
    xt2 = np.stack([
        xp.transpose(2, 0, 1).reshape(NLON, CIN * KP).reshape(4, 128, CIN * KP)
        for xp in (xe, xo)]).astype(BF)

    # W_a = [wr ; wi], W_b = [-wi ; wr] interleaved over (o,h) cols, per (s,j)
    wcm = np.zeros((64, 2, 128, 2, 128), np.float32)
    for s in range(2):
        wl = wr[:, :, s::2].transpose(0, 2, 1)      # [i, j, o]
        il = wi[:, :, s::2].transpose(0, 2, 1)
        wcm[:, s, :, 0, :] = np.stack([wl, il], axis=-1).reshape(64, 128, 128)
        wcm[:, s, :, 1, :] = np.stack([-il, wl], axis=-1).reshape(64, 128, 128)
    # stack (R,I) contraction rows: wc2[s, j] = [[Wa], [Wb]]  -> [128, 128]
    wc2 = wcm.transpose(1, 2, 3, 0, 4).reshape(2, 128, 128, 128).astype(BF)

    identb = np.eye(128, dtype=np.float32).astype(BF)
    n = np.arange(NLON)
    js = np.arange(128)

    in_maps = []
    for r in range(N_CORES):
        slots = _core_slots(r)
        fdft = np.zeros((NLON, MC), np.float32)
        gdft = np.zeros((MC, NLON), np.float32)
        shtw_p = np.zeros((M_LOC, KP, 2, 128), np.float32)
        pctw_p = np.zeros((M_LOC, 128, 4, KP), np.float32)
        for mi, m in enumerate(slots):
            if m is None:
                continue
            ang = 2.0 * np.pi * m * n / NLON
            fdft[:, mi] = (2.0 * np.pi / NLON) * np.cos(ang)
            fdft[:, M_LOC + mi] = -(2.0 * np.pi / NLON) * np.sin(ang)
            cmf = 1.0 if (m == 0 or m == NLON // 2) else 2.0
            gdft[2 * mi] = cmf * np.cos(ang)
            gdft[2 * mi + 1] = -cmf * np.sin(ang)
            for par in range(2):
                cls = (m + par) % 2     # l-parity class in psum half `par`
                shtw_p[mi, :, par, :] = sht_w[m, 2 * js + cls, :KP].T
            sgn = 1.0 if m % 2 == 0 else -1.0
            pctw_p[mi, :, 0, :] = pct[m, 2 * js, :KP]
            pctw_p[mi, :, 1, :] = pct[m, 2 * js + 1, :KP]
            pctw_p[mi, :, 2, :] = sgn * pct[m, 2 * js, :KP]
            pctw_p[mi, :, 3, :] = -sgn * pct[m, 2 * js + 1, :KP]

        in_maps.append({
            "xt2": xt2, "fdft": fdft.reshape(4, 128, MC).astype(BF),
            "shtw": shtw_p.astype(BF), "wc": wc2,
            "pctw": pctw_p.astype(BF),
            "gdft": gdft.astype(BF), "ident": identb,
        })
    return in_maps


def kernel(x, weight_r, weight_i, pct, sht_w):
    x_np = np.asarray(x)
    nc = _get_nc()
    in_maps = make_in_maps(x_np, weight_r, weight_i, pct, sht_w)
    try:
        res = run_bass_kernel_spmd(nc, in_maps, list(range(N_CORES)))
    except Exception:
        # transient NRT exec faults have been observed on the first run
        # after a NEFF load; one retry has always succeeded
        res = run_bass_kernel_spmd(nc, in_maps, list(range(N_CORES)))
    y = np.zeros((64, NLAT, NLON), np.float64)
    for r in range(N_CORES):
        yp = np.asarray(res.results[r]["y_part"], dtype=np.float64)
        yp = yp.reshape(2, 64, KP, NLON)
        y[:, :KP, :] += yp[0]
        y[:, KP:, :] += yp[1][:, ::-1, :]
    y = y.astype(np.float32).reshape(1, COUT_, NLAT, NLON)
    return (y, x_np)
